# revision 1
# baseline (speedup 1.0000x reference)
"""CenterLoss (segment-reduce) kernel for Trainium2, 8 NeuronCores.

Math: out = (1/B) * sum_j sums_j / (counts_j * F)  over classes j with
counts_j > 0, where sums_j = sum_{i: label_i=j} ||feat_i - center_j||^2.

Two device algorithms (CL_ALGO):

"dot" (default): expand ||f-c||^2 = ||f||^2 - 2<f,c> + ||c||^2 and fold the
  per-class weights on the host:
      out = [ sum_i w_i * (||f_i||^2 - 2<f_i, c_{l_i}>)
              + sum_{j: count_j>0} ||c_j||^2 ] / (F * B),   w_i = 1/count_{l_i}
  counts (and so w), plus the ||c_j||^2 term, are host-side numpy from
  labels/centers. The device only produces the two per-sample scalars:
      s2_i = ||f_i||^2       (ACT square + free-dim accumulate, or DVE)
      fc_i = <f_i, c_{l_i}>  (DVE tensor_tensor_reduce)
  Features stream in as [128 part, blk, 512] tiles; the matching center row
  for every sample is fetched with the GPSIMD dma_gather ucode instruction
  (SWDGE) into the identical layout, so both reductions are straight
  elementwise+accumulate ops with no data shuffling. No segment reduce on
  device at all.

"diff": original form — per-sample d_i = ||f_i - c_{l_i}||^2 via DVE
  subtract + ACT/DVE square-accumulate, then an on-device segment reduce
  into 1024 = 32x32 class bins with a factorized one-hot (class = 32q + r)
  and one PE matmul per 128-sample block accumulating into a [32,32] PSUM
  tile. counts still come from host bincount.
"""

import os
from contextlib import ExitStack

import numpy as np

import concourse.bacc as bacc
import concourse.bass as bass
import concourse.tile as tile
from concourse import mybir
from concourse.bass_utils import run_bass_kernel_spmd

NCORES = 8
BATCH = 65536
FEAT = 512
NCLASS = 1000
SHARD = BATCH // NCORES  # 8192
P = 128
NBLK = SHARD // P  # 64
CHUNK_BLKS = int(os.environ.get("CL_CHUNK_BLKS", "8"))  # blocks per DMA chunk
NCHUNK = NBLK // CHUNK_BLKS
DMA_BUFS = int(os.environ.get("CL_DMA_BUFS", "3"))
GBUFS = int(os.environ.get("CL_GBUFS", "0")) or DMA_BUFS
QW = 32  # diff algo: class = QW*q + r; 32*32 = 1024 bins >= 1000

ALGO = os.environ.get("CL_ALGO", "diff")  # "diff" | "dot"
# Dtype knobs: "f32" or "bf16" for the streamed features / gathered centers.
FEAT_DT = os.environ.get("CL_FEAT_DT", "bf16")
CENT_DT = os.environ.get("CL_CENT_DT", "bf16")
# How many of the blocks per chunk run the square-accumulate on ACT
# (the rest run on DVE) — balances the two engines.
ACT_BLOCKS = int(os.environ.get("CL_ACT_BLOCKS", "6"))
# Batched one-hot build (broadcast APs) vs per-block tensor_scalar ops.
BATCH_ONEHOT = os.environ.get("CL_BATCH_ONEHOT", "1") == "1"
# Spread chunk gathers across SWDGE queues (0 = all on queue 0).
GQ_SPREAD = min(int(os.environ.get("CL_GQ_SPREAD", "4")), 4)
# Split each chunk's gather into N sub-gathers on distinct SWDGE queues.
GSPLIT = int(os.environ.get("CL_GSPLIT", "2"))
# Issue feature DMAs alternately from N HWDGE engines (sync, scalar).
FDMA_SPREAD = min(int(os.environ.get("CL_FDMA_SPREAD", "2")), 2)
# Split the per-chunk d-weighting mult into N pieces for finer PE overlap.
MSPLIT = int(os.environ.get("CL_MSPLIT", "1"))

TRACE = os.environ.get("CL_TRACE", "0") == "1"
# timing-only ablations for TimelineSim analysis (comma list:
# feat,gather,sub,dsq)
ABLATE = set(filter(None, os.environ.get("CL_ABLATE", "").split(",")))

_DT = {"f32": mybir.dt.float32, "bf16": mybir.dt.bfloat16}


def _np_dt(name):
    if name == "f32":
        return np.float32
    import ml_dtypes

    return ml_dtypes.bfloat16


def _bcast_ap(ap, dims):
    """Build a broadcast AP from a 2-D tile AP [P, n]: dims is a list of
    ("b", count) for broadcast (stride 0) or ("d", count) to consume the
    tile's free dim."""
    part = ap.ap[0]
    free = ap.ap[1:]
    assert len(free) == 1
    stride = free[0][0]
    out = [part]
    for kind, count in dims:
        if kind == "b":
            out.append([0, count])
        else:
            out.append([stride, count])
    return bass.AP(tensor=ap.tensor, offset=ap.offset, ap=out)


def build_module(repeat: int = 1):
    if ALGO == "dot":
        return _build_dot(repeat)
    return _build_diff(repeat)


def _build_dot(repeat: int = 1):
    """Dot-form kernel: outputs per-sample s2 and fc, [128, 2*64] packed."""
    f32 = mybir.dt.float32
    i16 = mybir.dt.int16
    fdt = _DT[FEAT_DT]
    cdt = _DT[CENT_DT]
    ddt = fdt if fdt == cdt else f32  # scratch dtype

    nc = bacc.Bacc(
        "TRN2", target_bir_lowering=False, debug=False, num_devices=NCORES,
        num_swdge_queues=max(1, GQ_SPREAD),
    )
    feat_d = nc.dram_tensor("features", [SHARD, FEAT], fdt, kind="ExternalInput")
    cent_d = nc.dram_tensor("centers", [NCLASS, FEAT], cdt, kind="ExternalInput")
    idx_d = nc.dram_tensor("labels16", [P, SHARD // 16], i16, kind="ExternalInput")
    out_d = nc.dram_tensor("out", [P, 2 * NBLK], f32, kind="ExternalOutput")

    with tile.TileContext(nc) as tc:
        with ExitStack() as ctx:
            singles = ctx.enter_context(tc.tile_pool(name="singles", bufs=1))
            fpool = ctx.enter_context(tc.tile_pool(name="fpool", bufs=DMA_BUFS))
            gpool = ctx.enter_context(tc.tile_pool(name="gpool", bufs=GBUFS))
            sqpool = ctx.enter_context(tc.tile_pool(name="sqpool", bufs=4))
            fcpool = ctx.enter_context(tc.tile_pool(name="fcpool", bufs=4))

            idx_t = singles.tile([P, SHARD // 16], i16)
            nc.sync.dma_start(out=idx_t[:], in_=idx_d.ap())

            # out columns 0:64 = s2 = ||f||^2, 64:128 = fc = <f, c_label>
            res_t = singles.tile([P, 2 * NBLK], f32)
            feat_ap = feat_d.ap().rearrange("(b p) f -> p b f", p=P)

            if repeat > 1:
                loop_cm = tc.For_i(0, repeat, 1)
                loop_cm.__enter__()

            nidx = CHUNK_BLKS * P
            for c in range(NCHUNK):
                cs = slice(c * CHUNK_BLKS, (c + 1) * CHUNK_BLKS)
                ft = fpool.tile([P, CHUNK_BLKS, FEAT], fdt)
                if "feat" not in ABLATE:
                    nc.sync.dma_start(out=ft[:], in_=feat_ap[:, cs, :])
                else:
                    nc.vector.memset(ft[:, 0, 0:8], 0)
                gt = gpool.tile([P, CHUNK_BLKS, FEAT], cdt)
                if "gather" not in ABLATE:
                    nc.gpsimd.dma_gather(
                        out_ap=gt[:],
                        in_ap=cent_d.ap(),
                        idxs_ap=idx_t[
                            :, c * (nidx // 16) : (c + 1) * (nidx // 16)
                        ],
                        num_idxs=nidx,
                        num_idxs_reg=nidx,
                        elem_size=FEAT,
                        queue_num=(c % GQ_SPREAD) if GQ_SPREAD else 0,
                    )
                else:
                    nc.vector.memset(gt[:, 0, 0:8], 0)
                for j in range(CHUNK_BLKS):
                    b = c * CHUNK_BLKS + j
                    # s2 = ||f||^2 (only needs the feature tile)
                    if "dsq" not in ABLATE:
                        sq = sqpool.tile([P, FEAT], ddt)
                        if j < ACT_BLOCKS:
                            nc.scalar.activation(
                                out=sq[:],
                                in_=ft[:, j, :],
                                func=mybir.ActivationFunctionType.Square,
                                accum_out=res_t[:, b : b + 1],
                            )
                        else:
                            nc.vector.scalar_tensor_tensor(
                                out=sq[:],
                                in0=ft[:, j, :],
                                scalar=0.0,
                                in1=ft[:, j, :],
                                op0=mybir.AluOpType.bypass,
                                op1=mybir.AluOpType.mult,
                                accum_out=res_t[:, b : b + 1],
                            )
                    # fc = <f, c_label>
                    if "sub" not in ABLATE:
                        fcs = fcpool.tile([P, FEAT], ddt)
                        nc.vector.scalar_tensor_tensor(
                            out=fcs[:],
                            in0=ft[:, j, :],
                            scalar=0.0,
                            in1=gt[:, j, :],
                            op0=mybir.AluOpType.bypass,
                            op1=mybir.AluOpType.mult,
                            accum_out=res_t[:, NBLK + b : NBLK + b + 1],
                        )
            if ABLATE:
                nc.vector.memset(res_t[:, 0:1], 0)
            nc.sync.dma_start(out=out_d.ap(), in_=res_t[:])

            if repeat > 1:
                loop_cm.__exit__(None, None, None)

    nc.compile()
    return nc


def _build_diff(repeat: int = 1):
    """Original diff-form kernel with on-device factorized segment reduce."""
    f32 = mybir.dt.float32
    i16 = mybir.dt.int16
    fdt = _DT[FEAT_DT]
    cdt = _DT[CENT_DT]
    ddt = fdt if fdt == cdt else f32  # diff/square scratch dtype
    sdt = f32  # one-hot / rhs dtype (precision: keep f32)

    nc = bacc.Bacc(
        "TRN2", target_bir_lowering=False, debug=False, num_devices=NCORES,
        num_swdge_queues=max(1, GQ_SPREAD),
    )
    feat_d = nc.dram_tensor("features", [SHARD, FEAT], fdt, kind="ExternalInput")
    cent_d = nc.dram_tensor("centers", [NCLASS, FEAT], cdt, kind="ExternalInput")
    idx_d = nc.dram_tensor("labels16", [P, SHARD // 16], i16, kind="ExternalInput")
    q_d = nc.dram_tensor("qcol", [P, NBLK], f32, kind="ExternalInput")
    r_d = nc.dram_tensor("rcol", [P, NBLK], f32, kind="ExternalInput")
    iota_d = nc.dram_tensor("iota", [P, QW], sdt, kind="ExternalInput")
    out_d = nc.dram_tensor("out", [QW, QW], f32, kind="ExternalOutput")

    with tile.TileContext(nc) as tc:
        with ExitStack() as ctx:
            singles = ctx.enter_context(tc.tile_pool(name="singles", bufs=1))
            fpool = ctx.enter_context(tc.tile_pool(name="fpool", bufs=DMA_BUFS))
            gpool = ctx.enter_context(tc.tile_pool(name="gpool", bufs=GBUFS))
            dpool = ctx.enter_context(tc.tile_pool(name="dpool", bufs=4))
            sqpool = ctx.enter_context(tc.tile_pool(name="sqpool", bufs=4))
            small = ctx.enter_context(tc.tile_pool(name="small", bufs=4))
            psum_p = ctx.enter_context(
                tc.tile_pool(name="psum", bufs=1, space="PSUM")
            )

            idx_t = singles.tile([P, SHARD // 16], i16)
            nc.sync.dma_start(out=idx_t[:], in_=idx_d.ap())
            q_t = singles.tile([P, NBLK], f32)
            nc.sync.dma_start(out=q_t[:], in_=q_d.ap())
            r_t = singles.tile([P, NBLK], f32)
            nc.sync.dma_start(out=r_t[:], in_=r_d.ap())
            iota_t = singles.tile([P, QW], sdt)
            nc.sync.dma_start(out=iota_t[:], in_=iota_d.ap())

            if BATCH_ONEHOT:
                # one-hot(q) for all blocks: [P, b, j] = (iota[j] == q[p, b])
                ohq_all = singles.tile([P, NBLK, QW], sdt)
                nc.vector.tensor_tensor(
                    out=ohq_all[:],
                    in0=_bcast_ap(iota_t[:], [("b", NBLK), ("d", QW)]),
                    in1=_bcast_ap(q_t[:], [("d", NBLK), ("b", QW)]),
                    op=mybir.AluOpType.is_equal,
                )
                ohr_all = singles.tile([P, NBLK, QW], sdt)
                nc.vector.tensor_tensor(
                    out=ohr_all[:],
                    in0=_bcast_ap(iota_t[:], [("b", NBLK), ("d", QW)]),
                    in1=_bcast_ap(r_t[:], [("d", NBLK), ("b", QW)]),
                    op=mybir.AluOpType.is_equal,
                )
                # d-weighted one-hot(r), filled per chunk
                rhs_all = singles.tile([P, NBLK, QW], sdt)

            psum_t = psum_p.tile([QW, QW], f32, space="PSUM")
            feat_ap = feat_d.ap().rearrange("(b p) f -> p b f", p=P)

            if repeat > 1:
                loop_cm = tc.For_i(0, repeat, 1)
                loop_cm.__enter__()

            nidx = CHUNK_BLKS * P  # gather indices per chunk
            for c in range(NCHUNK):
                cs = slice(c * CHUNK_BLKS, (c + 1) * CHUNK_BLKS)
                ft = fpool.tile([P, CHUNK_BLKS, FEAT], fdt)
                fengines = [nc.sync, nc.scalar, nc.vector][:FDMA_SPREAD]
                half = CHUNK_BLKS // len(fengines)
                for e, eng in enumerate(fengines):
                    eng.dma_start(
                        out=ft[:, e * half : (e + 1) * half, :],
                        in_=feat_ap[
                            :,
                            c * CHUNK_BLKS + e * half : c * CHUNK_BLKS
                            + (e + 1) * half,
                            :,
                        ],
                    )
                gt = gpool.tile([P, CHUNK_BLKS, FEAT], cdt)
                gh = CHUNK_BLKS // GSPLIT
                for g in range(GSPLIT):
                    sidx = nidx // GSPLIT
                    nc.gpsimd.dma_gather(
                        out_ap=gt[:, g * gh : (g + 1) * gh, :],
                        in_ap=cent_d.ap(),
                        idxs_ap=idx_t[
                            :,
                            c * (nidx // 16) + g * (sidx // 16) : c * (nidx // 16)
                            + (g + 1) * (sidx // 16),
                        ],
                        num_idxs=sidx,
                        num_idxs_reg=sidx,
                        elem_size=FEAT,
                        queue_num=((c * GSPLIT + g) % GQ_SPREAD)
                        if GQ_SPREAD
                        else 0,
                    )
                d_chunk = small.tile([P, CHUNK_BLKS], f32)
                for j in range(CHUNK_BLKS):
                    diff = dpool.tile([P, FEAT], ddt)
                    nc.vector.tensor_tensor(
                        out=diff[:],
                        in0=ft[:, j, :],
                        in1=gt[:, j, :],
                        op=mybir.AluOpType.subtract,
                    )
                    sq = sqpool.tile([P, FEAT], ddt)
                    if j < ACT_BLOCKS:
                        nc.scalar.activation(
                            out=sq[:],
                            in_=diff[:],
                            func=mybir.ActivationFunctionType.Square,
                            accum_out=d_chunk[:, j : j + 1],
                        )
                    else:
                        nc.vector.scalar_tensor_tensor(
                            out=sq[:],
                            in0=diff[:],
                            scalar=0.0,
                            in1=diff[:],
                            op0=mybir.AluOpType.bypass,
                            op1=mybir.AluOpType.mult,
                            accum_out=d_chunk[:, j : j + 1],
                        )
                if BATCH_ONEHOT:
                    # rhs[:, b, :] = one-hot(r)[:, b, :] * d[:, b]
                    mh = CHUNK_BLKS // MSPLIT
                    for m in range(MSPLIT):
                        ms = slice(
                            c * CHUNK_BLKS + m * mh,
                            c * CHUNK_BLKS + (m + 1) * mh,
                        )
                        nc.vector.tensor_tensor(
                            out=rhs_all[:, ms, :],
                            in0=ohr_all[:, ms, :],
                            in1=_bcast_ap(
                                d_chunk[:, m * mh : (m + 1) * mh],
                                [("d", mh), ("b", QW)],
                            ),
                            op=mybir.AluOpType.mult,
                        )
                for j in range(CHUNK_BLKS):
                    b = c * CHUNK_BLKS + j
                    if BATCH_ONEHOT:
                        lhsT = ohq_all[:, b, :]
                        rhs = rhs_all[:, b, :]
                    else:
                        ohq_tile = small.tile([P, QW], sdt, tag=f"oq{j % 4}")
                        nc.vector.tensor_scalar(
                            out=ohq_tile[:],
                            in0=iota_t[:],
                            scalar1=q_t[:, b : b + 1],
                            scalar2=None,
                            op0=mybir.AluOpType.is_equal,
                        )
                        rhs_tile = small.tile([P, QW], sdt, tag=f"rh{j % 4}")
                        nc.vector.tensor_scalar(
                            out=rhs_tile[:],
                            in0=iota_t[:],
                            scalar1=r_t[:, b : b + 1],
                            scalar2=d_chunk[:, j : j + 1],
                            op0=mybir.AluOpType.is_equal,
                            op1=mybir.AluOpType.mult,
                        )
                        lhsT = ohq_tile[:]
                        rhs = rhs_tile[:]
                    nc.tensor.matmul(
                        out=psum_t[:],
                        lhsT=lhsT,
                        rhs=rhs,
                        start=(b == 0),
                        stop=(b == NBLK - 1),
                    )
            res_t = singles.tile([QW, QW], f32)
            nc.vector.tensor_copy(out=res_t[:], in_=psum_t[:])
            nc.sync.dma_start(out=out_d.ap(), in_=res_t[:])

            if repeat > 1:
                loop_cm.__exit__(None, None, None)

    nc.compile()
    return nc


_MODULE = None


def _get_module():
    global _MODULE
    if _MODULE is None:
        _MODULE = build_module()
    return _MODULE


def make_in_maps(features, centers, labels):
    """Host-side shard + layout prep. Returns list of 8 per-core input maps."""
    fdt = _np_dt(FEAT_DT)
    cdt = _np_dt(CENT_DT)
    features = np.ascontiguousarray(np.asarray(features), dtype=np.float32)
    centers = np.ascontiguousarray(np.asarray(centers), dtype=np.float32)
    labels = np.asarray(labels).astype(np.int64, copy=False)
    if fdt is not np.float32:
        features = features.astype(fdt)
    if cdt is not np.float32:
        centers = centers.astype(cdt)

    iota = np.ascontiguousarray(
        np.broadcast_to(np.arange(QW, dtype=np.float32), (P, QW))
    )
    in_maps = []
    for c in range(NCORES):
        lab = labels[c * SHARD : (c + 1) * SHARD]
        # wrapped-16 gather index layout: idx16[i % 16, i // 16] = lab[i],
        # replicated across the 8 groups of 16 partitions.
        idx16 = np.ascontiguousarray(lab.reshape(SHARD // 16, 16).T).astype(np.int16)
        idx16 = np.ascontiguousarray(np.tile(idx16, (8, 1)))
        m = {
            "features": features[c * SHARD : (c + 1) * SHARD],
            "centers": centers,
            "labels16": idx16,
        }
        if ALGO == "diff":
            lab_blk = lab.reshape(NBLK, P).T  # [p, b] = lab[b*128+p]
            m["qcol"] = np.ascontiguousarray((lab_blk // QW).astype(np.float32))
            m["rcol"] = np.ascontiguousarray((lab_blk % QW).astype(np.float32))
            m["iota"] = iota
        in_maps.append(m)
    return in_maps


def reduce_outputs(outs, labels, centers):
    """Combine per-core device partials + host-side terms into the loss."""
    labels = np.asarray(labels).astype(np.int64, copy=False)
    counts = np.bincount(labels, minlength=NCLASS)[:NCLASS]
    if ALGO == "diff":
        tot = np.sum(np.asarray(outs, dtype=np.float64), axis=0)  # [32, 32]
        sums = tot.reshape(-1)[:NCLASS]
        per_class = np.where(
            counts > 0, sums / np.maximum(counts * FEAT, 1.0), 0.0
        )
        return np.asarray(per_class.sum() / BATCH, dtype=np.float32)

    # dot algo: outs[c] is [P, 2*NBLK] = [s2 | fc] in block layout
    w = np.zeros(NCLASS)
    w[counts > 0] = 1.0 / counts[counts > 0]
    wi = w[labels]  # [B]
    t_parts = []
    for o in outs:
        o = np.asarray(o, dtype=np.float64)
        s2 = o[:, :NBLK].T.reshape(-1)  # sample i = b*128+p  -> [SHARD]
        fc = o[:, NBLK:].T.reshape(-1)
        t_parts.append(s2 - 2.0 * fc)
    t = np.concatenate(t_parts)  # [B], sample order
    cent64 = np.asarray(centers, dtype=np.float64)
    c2 = (cent64 * cent64).sum(axis=1)  # [NCLASS]
    total = (t * wi).sum() + c2[counts > 0].sum()
    return np.asarray(total / (FEAT * BATCH), dtype=np.float32)


LAST_RESULT = None


def kernel(features, centers, labels):
    global LAST_RESULT
    nc = _get_module()
    in_maps = make_in_maps(features, centers, labels)
    res = run_bass_kernel_spmd(
        nc, in_maps, core_ids=list(range(NCORES)), trace=TRACE
    )
    LAST_RESULT = res
    outs = [r["out"] for r in res.results]
    return reduce_outputs(outs, labels, np.asarray(centers, dtype=np.float32))

